# revision 1
# baseline (speedup 1.0000x reference)
"""CGRU cell on 8 Trainium2 NeuronCores.

Strategy: data-parallel over the batch dim (4096 -> 8 x 512). Each core
computes its h-shard with zero cross-core communication; weights are
replicated.

On-core compute runs in transposed space ([feature, batch]): the gate
pre-activations are (x @ W)^T = W^T @ x^T, so W tiles are the stationary
matmul operand and x^T/h^T tiles [128, 512] are the moving operand
(batch = 512 = one full fp32 PSUM bank).  The complex "cat kernel"
[[R, -I], [I, R]] is never materialized: real-out = R^Txr + I^Txi,
imag-out = R^Txi + (-I)^Txr, with R tiles shared by both outputs and
pre-negated I copies built on the host.  All matmuls are fp16 with fp32
PSUM accumulation; the final combine (z*h + (1-z)*hh) is fp32.

DMA queues: weights stream on the sync queue, activations load on the
gpsimd queue, outputs store on the scalar queue, so the first weight
tile is not stuck behind 6MB of activation loads.
"""

import numpy as np

import concourse.bass as bass
import concourse.mybir as mybir
import concourse.tile as tile
from concourse import bacc
from concourse.bass_utils import run_bass_kernel_spmd

B, D, U = 4096, 1024, 1024
NCORES = 8
N = B // NCORES          # batch rows per core (moving free dim)
P = 128                  # partition size
KT = D // P              # 8 k-tiles per complex half
MT = U // P              # 8 m-tiles per complex half
F = 2 * U                # 2048 features
MCOLS = KT * P           # 1024 cols per per-matrix weight tile

F16 = mybir.dt.float16
F32 = mybir.dt.float32
AF = mybir.ActivationFunctionType
OP = mybir.AluOpType

_CACHE = {}


def _build():
    nc = bacc.Bacc("TRN2", target_bir_lowering=False, debug=False)

    xT = nc.dram_tensor("xT", [F, N], F16, kind="ExternalInput")
    hT16 = nc.dram_tensor("hT16", [F, N], F16, kind="ExternalInput")
    hTf = nc.dram_tensor("hTf", [F, N], F32, kind="ExternalInput")
    w1 = nc.dram_tensor("w1", [MT, 2, 4, P, MCOLS], F16, kind="ExternalInput")
    w2 = nc.dram_tensor("w2", [MT, 4, P, MCOLS], F16, kind="ExternalInput")
    bzr = nc.dram_tensor("bzr", [P, 2, 2 * MT], F32, kind="ExternalInput")
    bh = nc.dram_tensor("bh", [P, 2 * MT], F32, kind="ExternalInput")
    oT = nc.dram_tensor("oT", [F, N], F32, kind="ExternalOutput")

    with tile.TileContext(nc) as tc:
        with (
            tc.tile_pool(name="res", bufs=1) as res,
            tc.tile_pool(name="wts", bufs=20) as wts,
            tc.tile_pool(name="act", bufs=3) as act,
            tc.tile_pool(name="ps", bufs=6, space="PSUM") as psp,
            tc.tile_pool(name="wm", bufs=1, space="PSUM") as wmp,
        ):
            # PE warmup: dummy matmuls on a zeroed tile keep the HAM
            # activity window busy while the first real DMAs land.
            wsrc = res.tile([P, P], F16, tag="wsrc")
            dmov = res.tile([P, N], F16, tag="dmov")
            nc.vector.memset(wsrc[:], 0.0)
            nc.vector.memset(dmov[:], 0.0)
            wps = wmp.tile([P, N], F32, tag="warm")
            for _ in range(10):
                nc.tensor.matmul(wps[:], wsrc[:], dmov[:], start=True, stop=True)

            xs = res.tile([P, 2 * MT, N], F16, tag="xs")
            hs = res.tile([P, 2 * MT, N], F16, tag="hs")
            hs32 = res.tile([P, 2 * MT, N], F32, tag="hs32")
            zs = res.tile([P, 2 * MT, N], F16, tag="zs")
            rh = res.tile([P, 2 * MT, N], F16, tag="rh")
            bz_sb = res.tile([P, 2, 2 * MT], F32, tag="bz")
            bh_sb = res.tile([P, 2 * MT], F32, tag="bh")

            nc.scalar.dma_start(bz_sb[:], bzr[:])
            nc.scalar.dma_start(bh_sb[:], bh[:])
            # (real, imag) interleaved so the k-ordered first accumulation
            # is fed in consumption order
            for k in range(MT):
                nc.gpsimd.dma_start(xs[:, k, :], xT[k * P:(k + 1) * P, :])
                nc.gpsimd.dma_start(xs[:, MT + k, :],
                                    xT[(MT + k) * P:(MT + k + 1) * P, :])
            for k in range(MT):
                nc.gpsimd.dma_start(hs[:, k, :], hT16[k * P:(k + 1) * P, :])
                nc.gpsimd.dma_start(hs[:, MT + k, :],
                                    hT16[(MT + k) * P:(MT + k + 1) * P, :])

            def load_w(src, ws=None, lo=0, hi=6):
                """six [P, MCOLS] weight tiles: R, I, -I, RR, IR, -IR.
                R/I/RR/IR stream from DRAM; -I and -IR are negated on-chip
                (saves a third of the weight DMA)."""
                if ws is None:
                    ws = [None] * 6
                for i in range(lo, hi):
                    if i in (2, 5):
                        wt = wts.tile([P, MCOLS], F16, tag="w")
                        nc.vector.tensor_scalar_mul(wt[:], ws[i - 1][:], -1.0)
                    else:
                        wt = wts.tile([P, MCOLS], F16, tag="w")
                        nc.sync.dma_start(wt[:], src[i - (i > 2)])
                    ws[i] = wt
                return ws

            def accum_half(ws, h, movs, ps_r, ps_i, kmajor=False):
                """32 matmuls: one (input or recurrent) half of a psum pair.

                kmajor interleaves the shared/I/-I matmuls per k-tile so PE
                consumption of freshly-DMA'd moving tiles matches arrival
                rate (used during the bandwidth-bound startup)."""
                sh_r, sh_i, only_r, only_i = movs
                if kmajor:
                    for k in range(KT):
                        wap = ws[h][:, k * P:(k + 1) * P]
                        nc.tensor.matmul(ps_r[:], wap, sh_r(k),
                                         start=(h == 0 and k == 0), stop=False)
                        nc.tensor.matmul(ps_i[:], wap, sh_i(k),
                                         start=(h == 0 and k == 0), stop=False)
                        nc.tensor.matmul(ps_r[:], ws[h + 1][:, k * P:(k + 1) * P],
                                         only_r(k), start=False,
                                         stop=(h == 3 and k == KT - 1))
                        nc.tensor.matmul(ps_i[:], ws[h + 2][:, k * P:(k + 1) * P],
                                         only_i(k), start=False,
                                         stop=(h == 3 and k == KT - 1))
                    return
                for k in range(KT):
                    wap = ws[h][:, k * P:(k + 1) * P]
                    nc.tensor.matmul(ps_r[:], wap, sh_r(k),
                                     start=(h == 0 and k == 0), stop=False)
                    nc.tensor.matmul(ps_i[:], wap, sh_i(k),
                                     start=(h == 0 and k == 0), stop=False)
                for k in range(KT):
                    nc.tensor.matmul(ps_r[:], ws[h + 1][:, k * P:(k + 1) * P],
                                     only_r(k),
                                     start=False, stop=(h == 3 and k == KT - 1))
                for k in range(KT):
                    nc.tensor.matmul(ps_i[:], ws[h + 2][:, k * P:(k + 1) * P],
                                     only_i(k),
                                     start=False, stop=(h == 3 and k == KT - 1))

            def accum_pair(ws, movs_a, movs_b, ps_r, ps_i):
                accum_half(ws, 0, movs_a, ps_r, ps_i)
                accum_half(ws, 3, movs_b, ps_r, ps_i)

            def xr(k):
                return xs[:, k, :]

            def xi(k):
                return xs[:, MT + k, :]

            def hr(k):
                return hs[:, k, :]

            def hi(k):
                return hs[:, MT + k, :]

            def rhr(k):
                return rh[:, k, :]

            def rhi(k):
                return rh[:, MT + k, :]

            # movs per weight triple: (shared->real, shared->imag,
            # I->real-only, -I->imag-only)
            movs_x = (xr, xi, xi, xr)
            movs_h = (hr, hi, hi, hr)
            movs_rh = (rhr, rhi, rhi, rhr)

            # ---- phase 1: z and r gates, rh = clip(r)*h ----
            for p in range(MT):
                pair = {}
                if p == 0:
                    # first m-pair: x-side weight tiles and x-side halves of
                    # both gates first, so the PE only needs the xs stream
                    # (and 3MB of weights) while hs still loads
                    for g in range(2):
                        ps_r = psp.tile([P, N], F32, tag="ps")
                        ps_i = psp.tile([P, N], F32, tag="ps")
                        pair[g] = (load_w(w1[p, g], lo=0, hi=3), ps_r, ps_i)
                    for g in range(2):
                        load_w(w1[p, g], ws=pair[g][0], lo=3, hi=6)
                    for g in range(2):
                        ws, ps_r, ps_i = pair[g]
                        accum_half(ws, 0, movs_x, ps_r, ps_i, kmajor=True)
                else:
                    for g in range(2):
                        ps_r = psp.tile([P, N], F32, tag="ps")
                        ps_i = psp.tile([P, N], F32, tag="ps")
                        pair[g] = (load_w(w1[p, g]), ps_r, ps_i)
                for g in range(2):
                    ws, ps_r, ps_i = pair[g]
                    if p == 0:
                        accum_half(ws, 3, movs_h, ps_r, ps_i, kmajor=True)
                    else:
                        accum_pair(ws, movs_x, movs_h, ps_r, ps_i)
                    if g == 0:
                        nc.scalar.activation(zs[:, p, :], ps_r[:], AF.Relu,
                                             bias=bz_sb[:, 0, p:p + 1], scale=0.2)
                        nc.scalar.activation(zs[:, MT + p, :], ps_i[:], AF.Relu,
                                             bias=bz_sb[:, 0, MT + p:MT + p + 1],
                                             scale=0.2)
                    else:
                        rr = act.tile([P, N], F16, tag="rr")
                        ri = act.tile([P, N], F16, tag="rr")
                        nc.scalar.activation(rr[:], ps_r[:], AF.Relu,
                                             bias=bz_sb[:, 1, p:p + 1], scale=0.2)
                        nc.scalar.activation(ri[:], ps_i[:], AF.Relu,
                                             bias=bz_sb[:, 1, MT + p:MT + p + 1],
                                             scale=0.2)
                        nc.vector.scalar_tensor_tensor(
                            rh[:, p, :], rr[:], 1.0, hs[:, p, :],
                            op0=OP.min, op1=OP.mult)
                        nc.vector.scalar_tensor_tensor(
                            rh[:, MT + p, :], ri[:], 1.0, hs[:, MT + p, :],
                            op0=OP.min, op1=OP.mult)

            # h in fp32 (for the final combine) loads behind phase 1
            for j in range(2 * MT):
                nc.gpsimd.dma_start(hs32[:, j, :], hTf[j * P:(j + 1) * P, :])

            def accum_single(ws, movs_a, movs_b, ps, which, cols):
                """32 matmuls into one psum tile (one column region)."""
                off = 1 if which == 0 else 2
                for h, movs in ((0, movs_a), (3, movs_b)):
                    sh, only = movs[which], movs[2 + which]
                    for k in range(KT):
                        nc.tensor.matmul(ps[:, cols], ws[h][:, k * P:(k + 1) * P],
                                         sh(k)[:, cols],
                                         start=(h == 0 and k == 0), stop=False)
                    for k in range(KT):
                        nc.tensor.matmul(ps[:, cols],
                                         ws[h + off][:, k * P:(k + 1) * P],
                                         only(k)[:, cols],
                                         start=False, stop=(h == 3 and k == KT - 1))

            # ---- phase 2: hh gate + final combine ----
            for p in range(MT):
                ws = load_w(w2[p])
                ps_r = psp.tile([P, N], F32, tag="ps")
                ps_i = psp.tile([P, N], F32, tag="ps")
                if p == MT - 1:
                    # last pair: real-out fully, then imag-out in column
                    # halves so its combine overlaps the accumulation and
                    # only a half-width chain trails the final matmul
                    accum_single(ws, movs_x, movs_rh, ps_r, 0, slice(0, N))
                    accum_single(ws, movs_x, movs_rh, ps_i, 1, slice(0, N // 2))
                    accum_single(ws, movs_x, movs_rh, ps_i, 1, slice(N // 2, N))
                else:
                    accum_pair(ws, movs_x, movs_rh, ps_r, ps_i)
                nsplit = 2 if p == MT - 1 else 1
                for m, ps in ((p, ps_r), (MT + p, ps_i)):
                    for si in range(nsplit):
                        c = slice(si * N // nsplit, (si + 1) * N // nsplit)
                        t = act.tile([P, N], F32, tag="t")
                        nc.scalar.activation(t[:, c], ps[:, c], AF.Tanh,
                                             bias=bh_sb[:, m:m + 1])
                        d = act.tile([P, N], F32, tag="d")
                        nc.vector.scalar_tensor_tensor(
                            d[:, c], t[:, c], -1.0, hs32[:, m, c],
                            op0=OP.mult, op1=OP.add)
                        e = act.tile([P, N], F32, tag="e")
                        nc.vector.scalar_tensor_tensor(
                            e[:, c], zs[:, m, c], 1.0, d[:, c],
                            op0=OP.min, op1=OP.mult)
                        o = act.tile([P, N], F32, tag="o")
                        nc.vector.tensor_tensor(o[:, c], e[:, c], t[:, c], OP.add)
                        nc.scalar.dma_start(oT[m * P:(m + 1) * P, c], o[:, c])

    nc.compile()
    return nc


def _tiles(mat):
    # (1024, 1024) -> [p, k, 128, 128] tile array
    return mat.reshape(KT, P, MT, P).transpose(2, 0, 1, 3)


def _gate_blob(mats):
    """[p, 4, 128, MCOLS] fp16 weight blob from (R, I, RR, IR)."""
    arr = np.stack([_tiles(m) for m in mats])  # [4, p, k, part, col]
    arr = arr.transpose(1, 0, 3, 2, 4)         # [p, mat, part, k, col]
    return arr.reshape(MT, 4, P, MCOLS).astype(np.float16)


def prepare_in_maps(inputs, h_tm1, real_kernel, imaginary_kernel,
                    real_recurrent_kernel, imaginary_recurrent_kernel,
                    real_bias, imaginary_bias):
    inputs = np.asarray(inputs, dtype=np.float32)
    h_tm1 = np.asarray(h_tm1, dtype=np.float32)

    def gate(Wmat, g):
        return np.asarray(Wmat[:, g * U:(g + 1) * U], dtype=np.float32)

    def mats(g):
        R, I = gate(real_kernel, g), gate(imaginary_kernel, g)
        RR, IR = gate(real_recurrent_kernel, g), gate(imaginary_recurrent_kernel, g)
        return (R, I, RR, IR)

    w1_np = np.ascontiguousarray(
        np.stack([_gate_blob(mats(0)), _gate_blob(mats(1))], axis=1))
    w2_np = np.ascontiguousarray(_gate_blob(mats(2)))

    def cat_bias(g):
        return np.concatenate([
            np.asarray(real_bias[g * U:(g + 1) * U], dtype=np.float32),
            np.asarray(imaginary_bias[g * U:(g + 1) * U], dtype=np.float32),
        ])

    bzr_np = np.ascontiguousarray(np.stack(
        [0.2 * cat_bias(g) + 0.5 for g in range(2)]).reshape(2, 2 * MT, P)
        .transpose(2, 0, 1))
    bh_np = np.ascontiguousarray(cat_bias(2).reshape(2 * MT, P).T)

    in_maps = []
    for c in range(NCORES):
        sl = slice(c * N, (c + 1) * N)
        hT = np.ascontiguousarray(h_tm1[sl].T)
        in_maps.append({
            "xT": inputs[sl].T.astype(np.float16),
            "hT16": hT.astype(np.float16),
            "hTf": hT,
            "w1": w1_np, "w2": w2_np, "bzr": bzr_np, "bh": bh_np,
        })
    return in_maps


def get_nc():
    if "nc" not in _CACHE:
        _CACHE["nc"] = _build()
    return _CACHE["nc"]


def gather(results):
    out = np.empty((B, F), dtype=np.float32)
    for c in range(NCORES):
        out[c * N:(c + 1) * N] = res_oT(results, c)
    return out


def res_oT(results, c):
    return results[c]["oT"].T


def kernel(**inputs):
    nc = get_nc()
    in_maps = prepare_in_maps(**inputs)
    res = run_bass_kernel_spmd(nc, in_maps, list(range(NCORES)))
    return gather(res.results)



# revision 10
# speedup vs baseline: 1.4982x; 1.4982x over previous
"""CGRU cell on 8 Trainium2 NeuronCores.

Strategy: data-parallel over the batch dim (4096 -> 8 x 512), zero
cross-core communication, weights replicated.  On-core compute runs in
transposed space ([feature, batch]): W tiles are the stationary matmul
operand, x^T/h^T tiles [128, 512] the moving operand.

The complex "cat kernel" [[R, -I], [I, R]] is evaluated with Gauss's
3-multiplication trick instead of 4 real matmul chains:
    k1   = (xr + xi) @ R        (shared by both outputs)
    real = k1 + xi @ (I - R)
    imag = k1 + xr @ (-I - R)
which cuts PE work by 25%.  The combine (k1 + A) is a single DVE
scalar_tensor_tensor reading two PSUM banks, fused with the per-feature
bias; hard-sigmoid scale/clip ride the scalar-engine activation.

The r gate (least error-sensitive: its error passes through another
matmul and a tanh) runs in fp8-e4m3 with DoubleRow perf mode (2x PE
throughput, two k-tiles contracted per instruction).  Its weights are
host-scaled by S8=64 to stay out of the fp8 subnormal range; the
activation's scale folds 0.2/S8 back out.  z and hh stay fp16.

Everything accumulates in fp32 PSUM; the final combine reads fp16 h.
"""

import numpy as np
import ml_dtypes

import concourse.bass as bass
import concourse.mybir as mybir
import concourse.tile as tile
from concourse import bacc
from concourse.bass_utils import run_bass_kernel_spmd

B, D, U = 4096, 1024, 1024
NCORES = 8
N = B // NCORES          # batch rows per core (moving free dim)
P = 128                  # partition size
KT = D // P              # 8 k-tiles per complex half
MT = U // P              # 8 m-tiles per complex half
F = 2 * U                # 2048 features
MCOLS = KT * P           # 1024 cols per per-matrix weight tile
S8 = 64.0                # fp8 weight pre-scale for the r gate
WARM = 12

F16 = mybir.dt.float16
F32 = mybir.dt.float32
F8 = mybir.dt.float8e4
AF = mybir.ActivationFunctionType
OP = mybir.AluOpType
DR = mybir.MatmulPerfMode.DoubleRow

_CACHE = {}


def _build():
    nc = bacc.Bacc("TRN2", target_bir_lowering=False, debug=False)

    xT = nc.dram_tensor("xT", [F, N], F16, kind="ExternalInput")
    hT = nc.dram_tensor("hT", [F, N], F16, kind="ExternalInput")
    # z/hh weights: [gate, m, mat, part, k*128] fp16,
    # mats: R, I-R, -(I+R), RR, IR-RR, -(IR+RR)
    w16 = nc.dram_tensor("w16", [2, MT, 6, P, MCOLS], F16, kind="ExternalInput")
    # r weights: same six mats, * S8, [m, mat, part, k, 128] fp8 (DoubleRow)
    w8 = nc.dram_tensor("w8", [MT, 6, P, KT, P], F8, kind="ExternalInput")
    bz = nc.dram_tensor("bz", [P, 2 * MT], F32, kind="ExternalInput")
    br = nc.dram_tensor("br", [P, 2 * MT], F32, kind="ExternalInput")
    bh = nc.dram_tensor("bh", [P, 2 * MT], F32, kind="ExternalInput")
    oT = nc.dram_tensor("oT", [F, N], F16, kind="ExternalOutput")

    with tile.TileContext(nc) as tc:
        with (
            tc.tile_pool(name="res", bufs=1) as res,
            tc.tile_pool(name="wts", bufs=18) as wts,
            tc.tile_pool(name="w8p", bufs=12) as w8p,
            tc.tile_pool(name="act", bufs=4) as act,
            tc.tile_pool(name="ps", bufs=6, space="PSUM") as psp,
            tc.tile_pool(name="wm", bufs=1, space="PSUM") as wmp,
        ):
            # PE warmup: keeps the HAM activity window busy while the
            # first DMAs land.
            wsrc = res.tile([P, P], F16, tag="wsrc")
            dmov = res.tile([P, N], F16, tag="dmov")
            nc.vector.memset(wsrc[:], 0.0)
            nc.vector.memset(dmov[:], 0.0)
            wps = wmp.tile([P, N], F32, tag="warm")
            for _ in range(WARM):
                nc.tensor.matmul(wps[:], wsrc[:], dmov[:], start=True, stop=True)

            xs = res.tile([P, 2 * MT, N], F16, tag="xs")      # xr 0..7, xi 8..15
            hs = res.tile([P, 2 * MT, N], F16, tag="hs")
            xsum = res.tile([P, KT, N], F16, tag="xsum")      # xr + xi
            hsum = res.tile([P, KT, N], F16, tag="hsum")
            x8 = res.tile([P, 2 * MT, N], F8, tag="x8")
            h8 = res.tile([P, 2 * MT, N], F8, tag="h8")
            xsum8 = res.tile([P, KT, N], F8, tag="xsum8")
            hsum8 = res.tile([P, KT, N], F8, tag="hsum8")
            rh = res.tile([P, 2 * MT, N], F16, tag="rh")      # min(r,1)*h
            rhsum = res.tile([P, KT, N], F16, tag="rhsum")
            zs = res.tile([P, 2 * MT, N], F16, tag="zs")
            bz_sb = res.tile([P, 2 * MT], F32, tag="bz")
            br_sb = res.tile([P, 2 * MT], F32, tag="br")
            bh_sb = res.tile([P, 2 * MT], F32, tag="bh")

            nc.scalar.dma_start(bz_sb[:], bz[:])
            nc.scalar.dma_start(br_sb[:], br[:])
            nc.scalar.dma_start(bh_sb[:], bh[:])

            # --- startup: one JIT-ordered DMA stream on the sync queue.
            # x pairs, z0/z1 weights (chain-use order 1,2,0 / 4,3,5) and h
            # pairs interleaved so arrival tracks PE consumption. ---
            def wload(ws, gi, m, j):
                wt = wts.tile([P, MCOLS], F16, tag="w")
                nc.sync.dma_start(wt[:], w16[gi, m, j])
                ws[j] = wt

            def xpair(k):
                nc.sync.dma_start(xs[:, MT + k, :],
                                  xT[(MT + k) * P:(MT + k + 1) * P, :])
                nc.sync.dma_start(xs[:, k, :], xT[k * P:(k + 1) * P, :])

            def hpair(k):
                nc.sync.dma_start(hs[:, MT + k, :],
                                  hT[(MT + k) * P:(MT + k + 1) * P, :])
                nc.sync.dma_start(hs[:, k, :], hT[k * P:(k + 1) * P, :])

            ws0 = [None] * 6
            ws1 = [None] * 6
            xpair(0)
            xpair(1)
            wload(ws0, 0, 0, 1)
            xpair(2)
            wload(ws0, 0, 0, 2)
            xpair(3)
            wload(ws0, 0, 0, 0)
            xpair(4)
            xpair(5)
            wload(ws1, 0, 1, 1)
            xpair(6)
            wload(ws1, 0, 1, 2)
            xpair(7)
            wload(ws1, 0, 1, 0)
            hpair(0)
            hpair(1)
            wload(ws0, 0, 0, 4)
            hpair(2)
            wload(ws0, 0, 0, 3)
            hpair(3)
            wload(ws0, 0, 0, 5)
            hpair(4)
            wload(ws1, 0, 1, 4)
            hpair(5)
            wload(ws1, 0, 1, 3)
            hpair(6)
            wload(ws1, 0, 1, 5)
            hpair(7)

            for k in range(KT):
                nc.vector.tensor_tensor(xsum[:, k, :], xs[:, k, :],
                                        xs[:, MT + k, :], OP.add)
            for k in range(KT):
                nc.vector.tensor_tensor(hsum[:, k, :], hs[:, k, :],
                                        hs[:, MT + k, :], OP.add)

            def emit_fp8_casts():
                for j in range(2 * MT):
                    nc.vector.tensor_scalar_mul(x8[:, j, :], xs[:, j, :], 1.0)
                for k in range(KT):
                    nc.vector.tensor_scalar_mul(xsum8[:, k, :], xsum[:, k, :], 1.0)
                for j in range(2 * MT):
                    nc.vector.tensor_scalar_mul(h8[:, j, :], hs[:, j, :], 1.0)
                for k in range(KT):
                    nc.vector.tensor_scalar_mul(hsum8[:, k, :], hsum[:, k, :], 1.0)

            class T:
                def __init__(self, kind, m, cols, ws=None, tail=False):
                    self.kind, self.m, self.cols, self.tail = kind, m, cols, tail
                    # alloc order (A, B, K1) matches in-half issue order so
                    # bank recycling waits line up with the combine stts
                    self.A = psp.tile([P, N], F32, tag="ps")
                    self.B = psp.tile([P, N], F32, tag="ps")
                    self.K1 = psp.tile([P, N], F32, tag="ps")
                    if ws is not None:
                        self.ws = ws
                        return
                    if kind == "r":
                        self.ws = []
                        for j in range(6):
                            wt = w8p.tile([P, KT, P], F8, tag="w8")
                            nc.sync.dma_start(wt[:], w8[m, j])
                            self.ws.append(wt)
                    else:
                        gi = 0 if kind == "z" else 1
                        self.ws = [None] * 6
                        for j in (1, 2, 0, 4, 3, 5):
                            wload(self.ws, gi, m, j)

            def dr_chain(t, ps, wj, buf, off, start):
                c = t.cols
                for j in range(KT // 2):
                    nc.tensor.matmul(
                        ps[:, c], t.ws[wj][:, 2 * j:2 * j + 2, :],
                        buf[:, off + 2 * j:off + 2 * j + 2, c],
                        start=(start and j == 0),
                        stop=(not start and j == KT // 2 - 1),
                        perf_mode=DR)

            def f16_chain(t, ps, wj, buf, off, start):
                c = t.cols
                for k in range(KT):
                    nc.tensor.matmul(
                        ps[:, c], t.ws[wj][:, k * P:(k + 1) * P],
                        buf[:, off + k, c],
                        start=(start and k == 0),
                        stop=(not start and k == KT - 1))

            def in_half(t, kmajor=False):
                c = t.cols
                if t.kind == "r":
                    dr_chain(t, t.A, 1, x8, MT, True)
                    dr_chain(t, t.B, 2, x8, 0, True)
                    dr_chain(t, t.K1, 0, xsum8, 0, True)
                elif kmajor:
                    # consume (xi[k], xr[k]) at DMA arrival rate
                    for k in range(KT):
                        nc.tensor.matmul(t.A[:, c],
                                         t.ws[1][:, k * P:(k + 1) * P],
                                         xs[:, MT + k, c],
                                         start=(k == 0), stop=False)
                        nc.tensor.matmul(t.B[:, c],
                                         t.ws[2][:, k * P:(k + 1) * P],
                                         xs[:, k, c],
                                         start=(k == 0), stop=False)
                    f16_chain(t, t.K1, 0, xsum, 0, True)
                else:
                    f16_chain(t, t.A, 1, xs, MT, True)
                    f16_chain(t, t.B, 2, xs, 0, True)
                    f16_chain(t, t.K1, 0, xsum, 0, True)

            def rec_and_combine(t):
                c, m = t.cols, t.m
                if t.kind == "r":
                    dr_chain(t, t.A, 4, h8, MT, False)
                    dr_chain(t, t.K1, 3, hsum8, 0, False)
                else:
                    mv, sm = (rh, rhsum) if t.kind == "h" else (hs, hsum)
                    f16_chain(t, t.A, 4, mv, MT, False)
                    f16_chain(t, t.K1, 3, sm, 0, False)
                # ALU ops may read only one PSUM operand: stage K1 in SBUF
                # (also releases its bank early)
                k1sb = act.tile([P, N], F32, tag="k1")
                nc.scalar.copy(k1sb[:, c], t.K1[:, c])
                pre_r = act.tile([P, N], F16, tag="pre")
                nc.vector.scalar_tensor_tensor(
                    pre_r[:, c], k1sb[:, c], 1.0, t.A[:, c],
                    op0=OP.mult, op1=OP.add)
                if t.kind == "r":
                    dr_chain(t, t.B, 5, h8, 0, False)
                else:
                    f16_chain(t, t.B, 5, mv, 0, False)
                pre_i = act.tile([P, N], F16, tag="pre")
                nc.vector.scalar_tensor_tensor(
                    pre_i[:, c], k1sb[:, c], 1.0, t.B[:, c],
                    op0=OP.mult, op1=OP.add)

                if t.kind == "z":
                    for col, pre in ((m, pre_r), (MT + m, pre_i)):
                        nc.scalar.activation(zs[:, col, c], pre[:, c], AF.Relu,
                                             bias=bz_sb[:, col:col + 1],
                                             scale=0.2)
                elif t.kind == "r":
                    for col, pre in ((m, pre_r), (MT + m, pre_i)):
                        rr = act.tile([P, N], F16, tag="rr")
                        nc.scalar.activation(rr[:, c], pre[:, c], AF.Relu,
                                             bias=br_sb[:, col:col + 1],
                                             scale=0.2 / S8)
                        nc.vector.scalar_tensor_tensor(
                            rh[:, col, c], rr[:, c], 1.0, hs[:, col, c],
                            op0=OP.min, op1=OP.mult)
                    nc.vector.tensor_tensor(rhsum[:, m, c], rh[:, m, c],
                                            rh[:, MT + m, c], OP.add)
                else:
                    for col, pre in ((m, pre_r), (MT + m, pre_i)):
                        # chunk the very last combine so its serial chain
                        # overlaps itself across engines
                        if t.tail and col >= MT:
                            chunks = (slice(0, N // 2), slice(N // 2, N))
                        else:
                            chunks = (c,)
                        for ch in chunks:
                            t_ = act.tile([P, N], F16, tag="t")
                            nc.scalar.activation(t_[:, ch], pre[:, ch],
                                                 AF.Tanh,
                                                 bias=bh_sb[:, col:col + 1])
                            d = act.tile([P, N], F16, tag="d")
                            nc.vector.scalar_tensor_tensor(
                                d[:, ch], t_[:, ch], -1.0, hs[:, col, ch],
                                op0=OP.mult, op1=OP.add)
                            e = act.tile([P, N], F16, tag="e")
                            nc.vector.scalar_tensor_tensor(
                                e[:, ch], zs[:, col, ch], 1.0, d[:, ch],
                                op0=OP.min, op1=OP.mult)
                            o = act.tile([P, N], F16, tag="o")
                            nc.vector.tensor_tensor(o[:, ch], e[:, ch],
                                                    t_[:, ch], OP.add)
                            nc.scalar.dma_start(oT[col * P:(col + 1) * P, ch],
                                                o[:, ch])

            full = slice(0, N)
            plan = ([("z", m) for m in range(4)]
                    + [("r", m) for m in range(MT)]
                    + [("z", m) for m in range(4, MT)]
                    + [("h", m) for m in range(MT)])

            prev = None
            for kind, m in plan:
                if kind == "r" and m == 0:
                    emit_fp8_casts()
                pre_ws = ws0 if (kind, m) == ("z", 0) else (
                    ws1 if (kind, m) == ("z", 1) else None)
                t = T(kind, m, full, ws=pre_ws,
                      tail=(kind == "h" and m == MT - 1))
                in_half(t, kmajor=(prev is None))
                if prev is not None:
                    rec_and_combine(prev)
                prev = t
            rec_and_combine(prev)

    nc.compile()
    return nc


def _tiles(mat):
    # (1024, 1024) -> [m, k, 128, 128] tile array
    return mat.reshape(KT, P, MT, P).transpose(2, 0, 1, 3)


def _gate_mats(real_kernel, imaginary_kernel, real_recurrent_kernel,
               imaginary_recurrent_kernel, g):
    def gate(Wmat):
        return np.asarray(Wmat[:, g * U:(g + 1) * U], dtype=np.float32)

    R, I = gate(real_kernel), gate(imaginary_kernel)
    RR, IR = gate(real_recurrent_kernel), gate(imaginary_recurrent_kernel)
    return [R, I - R, -(I + R), RR, IR - RR, -(IR + RR)]


def _blob16(mats):
    arr = np.stack([_tiles(m) for m in mats])   # [6, m, k, p, c]
    arr = arr.transpose(1, 0, 3, 2, 4)          # [m, mat, p, k, c]
    return arr.reshape(MT, 6, P, MCOLS).astype(np.float16)


def _blob8(mats):
    arr = np.stack([_tiles(m) for m in mats])
    arr = arr.transpose(1, 0, 3, 2, 4) * S8     # [m, mat, p, k, c]
    return np.ascontiguousarray(arr).astype(ml_dtypes.float8_e4m3)


def prepare_in_maps(inputs, h_tm1, real_kernel, imaginary_kernel,
                    real_recurrent_kernel, imaginary_recurrent_kernel,
                    real_bias, imaginary_bias):
    inputs = np.asarray(inputs, dtype=np.float32)
    h_tm1 = np.asarray(h_tm1, dtype=np.float32)

    def mats(g):
        return _gate_mats(real_kernel, imaginary_kernel,
                          real_recurrent_kernel, imaginary_recurrent_kernel, g)

    w16_np = np.ascontiguousarray(np.stack([_blob16(mats(0)),
                                            _blob16(mats(2))]))
    w8_np = _blob8(mats(1))

    def cat_bias(g):
        return np.concatenate([
            np.asarray(real_bias[g * U:(g + 1) * U], dtype=np.float32),
            np.asarray(imaginary_bias[g * U:(g + 1) * U], dtype=np.float32),
        ])

    def pcols(v):  # [2U] -> [P, 2MT]
        return np.ascontiguousarray(v.reshape(2 * MT, P).T)

    bz_np = pcols(0.2 * cat_bias(0) + 0.5)
    br_np = pcols(0.2 * cat_bias(1) + 0.5)
    bh_np = pcols(cat_bias(2))

    in_maps = []
    for c in range(NCORES):
        sl = slice(c * N, (c + 1) * N)
        in_maps.append({
            "xT": inputs[sl].T.astype(np.float16),
            "hT": h_tm1[sl].T.astype(np.float16),
            "w16": w16_np, "w8": w8_np,
            "bz": bz_np, "br": br_np, "bh": bh_np,
        })
    return in_maps


def get_nc():
    if "nc" not in _CACHE:
        _CACHE["nc"] = _build()
    return _CACHE["nc"]


def gather(results):
    out = np.empty((B, F), dtype=np.float32)
    for c in range(NCORES):
        out[c * N:(c + 1) * N] = results[c]["oT"].T.astype(np.float32)
    return out


def kernel(**inputs):
    nc = get_nc()
    in_maps = prepare_in_maps(**inputs)
    res = run_bass_kernel_spmd(nc, in_maps, list(range(NCORES)))
    return gather(res.results)


# revision 15
# speedup vs baseline: 1.5108x; 1.0084x over previous
"""CGRU cell on 8 Trainium2 NeuronCores.

Strategy: data-parallel over the batch dim (4096 -> 8 x 512), zero
cross-core communication, weights replicated.  On-core compute runs in
transposed space ([feature, batch]): W tiles are the stationary matmul
operand, x^T/h^T tiles [128, 512] the moving operand.

The complex "cat kernel" [[R, -I], [I, R]] is evaluated with Gauss's
3-multiplication trick instead of 4 real matmul chains:
    k1   = (xr + xi) @ R        (shared by both outputs)
    real = k1 + xi @ (I - R)
    imag = k1 + xr @ (-I - R)
which cuts PE work by 25%.  The combine (k1 + A) is a single DVE
scalar_tensor_tensor reading two PSUM banks, fused with the per-feature
bias; hard-sigmoid scale/clip ride the scalar-engine activation.

The r gate (least error-sensitive: its error passes through another
matmul and a tanh) runs in fp8-e4m3 with DoubleRow perf mode (2x PE
throughput, two k-tiles contracted per instruction).  Its weights are
host-scaled by S8=64 to stay out of the fp8 subnormal range; the
activation's scale folds 0.2/S8 back out.  z and hh stay fp16.

Everything accumulates in fp32 PSUM; the final combine reads fp16 h.
"""

import numpy as np
import ml_dtypes

import concourse.bass as bass
import concourse.mybir as mybir
import concourse.tile as tile
from concourse import bacc
from concourse.bass_utils import run_bass_kernel_spmd

B, D, U = 4096, 1024, 1024
NCORES = 8
N = B // NCORES          # batch rows per core (moving free dim)
P = 128                  # partition size
KT = D // P              # 8 k-tiles per complex half
MT = U // P              # 8 m-tiles per complex half
F = 2 * U                # 2048 features
MCOLS = KT * P           # 1024 cols per per-matrix weight tile
S8 = 64.0                # fp8 weight pre-scale for the r gate
WARM = 14

F16 = mybir.dt.float16
F32 = mybir.dt.float32
F8 = mybir.dt.float8e4
AF = mybir.ActivationFunctionType
OP = mybir.AluOpType
DR = mybir.MatmulPerfMode.DoubleRow

_CACHE = {}


def _build():
    nc = bacc.Bacc("TRN2", target_bir_lowering=False, debug=False)

    xT = nc.dram_tensor("xT", [F, N], F16, kind="ExternalInput")
    hT = nc.dram_tensor("hT", [F, N], F16, kind="ExternalInput")
    # z/hh weights: [gate, m, mat, part, k*128] fp16,
    # mats: R, I-R, -(I+R), RR, IR-RR, -(IR+RR)
    w16 = nc.dram_tensor("w16", [2, MT, 6, P, MCOLS], F16, kind="ExternalInput")
    # r weights: same six mats, * S8, [m, mat, part, k, 128] fp8 (DoubleRow)
    w8 = nc.dram_tensor("w8", [MT, 6, P, KT, P], F8, kind="ExternalInput")
    bz = nc.dram_tensor("bz", [P, 2 * MT], F32, kind="ExternalInput")
    br = nc.dram_tensor("br", [P, 2 * MT], F32, kind="ExternalInput")
    bh = nc.dram_tensor("bh", [P, 2 * MT], F32, kind="ExternalInput")
    oT = nc.dram_tensor("oT", [F, N], F16, kind="ExternalOutput")

    with tile.TileContext(nc) as tc:
        with (
            tc.tile_pool(name="res", bufs=1) as res,
            tc.tile_pool(name="wts", bufs=18) as wts,
            tc.tile_pool(name="w8p", bufs=12) as w8p,
            tc.tile_pool(name="act", bufs=4) as act,
            tc.tile_pool(name="ps", bufs=7, space="PSUM") as psp,
            tc.tile_pool(name="wm", bufs=1, space="PSUM") as wmp,
        ):
            # PE warmup: keeps the HAM activity window busy while the
            # first DMAs land.
            wsrc = res.tile([P, P], F16, tag="wsrc")
            dmov = res.tile([P, N], F16, tag="dmov")
            nc.vector.memset(wsrc[:], 0.0)
            nc.vector.memset(dmov[:], 0.0)
            wps = wmp.tile([P, N], F32, tag="warm")
            for _ in range(WARM):
                nc.tensor.matmul(wps[:], wsrc[:], dmov[:], start=True, stop=True)

            xs = res.tile([P, 2 * MT, N], F16, tag="xs")      # xr 0..7, xi 8..15
            hs = res.tile([P, 2 * MT, N], F16, tag="hs")
            xsum = res.tile([P, KT, N], F16, tag="xsum")      # xr + xi
            hsum = res.tile([P, KT, N], F16, tag="hsum")
            x8 = res.tile([P, 2 * MT, N], F8, tag="x8")
            h8 = res.tile([P, 2 * MT, N], F8, tag="h8")
            xsum8 = res.tile([P, KT, N], F8, tag="xsum8")
            hsum8 = res.tile([P, KT, N], F8, tag="hsum8")
            rh = res.tile([P, 2 * MT, N], F16, tag="rh")      # min(r,1)*h
            rhsum = res.tile([P, KT, N], F16, tag="rhsum")
            zs = res.tile([P, 2 * MT, N], F16, tag="zs")
            bz_sb = res.tile([P, 2 * MT], F32, tag="bz")
            br_sb = res.tile([P, 2 * MT], F32, tag="br")
            bh_sb = res.tile([P, 2 * MT], F32, tag="bh")

            nc.scalar.dma_start(bz_sb[:], bz[:])
            nc.scalar.dma_start(br_sb[:], br[:])
            nc.scalar.dma_start(bh_sb[:], bh[:])

            # --- startup: one JIT-ordered DMA stream on the sync queue.
            # x pairs, z0/z1 weights (chain-use order 1,2,0 / 4,3,5) and h
            # pairs interleaved so arrival tracks PE consumption. ---
            def wload(ws, gi, m, j):
                wt = wts.tile([P, MCOLS], F16, tag="w")
                nc.sync.dma_start(wt[:], w16[gi, m, j])
                ws[j] = wt

            def xpair(k):
                nc.sync.dma_start(xs[:, MT + k, :],
                                  xT[(MT + k) * P:(MT + k + 1) * P, :])
                nc.sync.dma_start(xs[:, k, :], xT[k * P:(k + 1) * P, :])

            def hpair(k):
                nc.sync.dma_start(hs[:, MT + k, :],
                                  hT[(MT + k) * P:(MT + k + 1) * P, :])
                nc.sync.dma_start(hs[:, k, :], hT[k * P:(k + 1) * P, :])

            ws0 = [None] * 6
            ws1 = [None] * 6
            xpair(0)
            xpair(1)
            wload(ws0, 0, 0, 1)
            xpair(2)
            wload(ws0, 0, 0, 2)
            xpair(3)
            wload(ws0, 0, 0, 0)
            xpair(4)
            xpair(5)
            wload(ws1, 0, 1, 1)
            xpair(6)
            wload(ws1, 0, 1, 2)
            xpair(7)
            wload(ws1, 0, 1, 0)
            hpair(0)
            hpair(1)
            wload(ws0, 0, 0, 4)
            hpair(2)
            wload(ws0, 0, 0, 3)
            hpair(3)
            wload(ws0, 0, 0, 5)
            hpair(4)
            wload(ws1, 0, 1, 4)
            hpair(5)
            wload(ws1, 0, 1, 3)
            hpair(6)
            wload(ws1, 0, 1, 5)
            hpair(7)

            for k in range(KT):
                nc.vector.tensor_tensor(xsum[:, k, :], xs[:, k, :],
                                        xs[:, MT + k, :], OP.add)
            for k in range(KT):
                nc.vector.tensor_tensor(hsum[:, k, :], hs[:, k, :],
                                        hs[:, MT + k, :], OP.add)

            def emit_fp8_casts():
                for j in range(2 * MT):
                    nc.vector.tensor_scalar_mul(x8[:, j, :], xs[:, j, :], 1.0)
                for k in range(KT):
                    nc.vector.tensor_scalar_mul(xsum8[:, k, :], xsum[:, k, :], 1.0)
                for j in range(2 * MT):
                    nc.vector.tensor_scalar_mul(h8[:, j, :], hs[:, j, :], 1.0)
                for k in range(KT):
                    nc.vector.tensor_scalar_mul(hsum8[:, k, :], hsum[:, k, :], 1.0)

            class T:
                def __init__(self, kind, m, cols, ws=None, tail=False):
                    self.kind, self.m, self.cols, self.tail = kind, m, cols, tail
                    # alloc order (A, B, K1) matches in-half issue order so
                    # bank recycling waits line up with the combine stts
                    self.A = psp.tile([P, N], F32, tag="ps")
                    self.B = psp.tile([P, N], F32, tag="ps")
                    self.K1 = psp.tile([P, N], F32, tag="ps")
                    if ws is not None:
                        self.ws = ws
                        return
                    if kind == "r":
                        self.ws = []
                        for j in range(6):
                            wt = w8p.tile([P, KT, P], F8, tag="w8")
                            nc.sync.dma_start(wt[:], w8[m, j])
                            self.ws.append(wt)
                    else:
                        gi = 0 if kind == "z" else 1
                        self.ws = [None] * 6
                        for j in (1, 2, 0, 4, 3, 5):
                            wload(self.ws, gi, m, j)

            def dr_chain(t, ps, wj, buf, off, start):
                c = t.cols
                for j in range(KT // 2):
                    nc.tensor.matmul(
                        ps[:, c], t.ws[wj][:, 2 * j:2 * j + 2, :],
                        buf[:, off + 2 * j:off + 2 * j + 2, c],
                        start=(start and j == 0),
                        stop=(not start and j == KT // 2 - 1),
                        perf_mode=DR)

            def f16_chain(t, ps, wj, buf, off, start):
                c = t.cols
                for k in range(KT):
                    nc.tensor.matmul(
                        ps[:, c], t.ws[wj][:, k * P:(k + 1) * P],
                        buf[:, off + k, c],
                        start=(start and k == 0),
                        stop=(not start and k == KT - 1))

            def in_half(t, kmajor=False):
                c = t.cols
                if t.kind == "r":
                    dr_chain(t, t.A, 1, x8, MT, True)
                    dr_chain(t, t.B, 2, x8, 0, True)
                    dr_chain(t, t.K1, 0, xsum8, 0, True)
                elif kmajor:
                    # consume (xi[k], xr[k]) at DMA arrival rate
                    for k in range(KT):
                        nc.tensor.matmul(t.A[:, c],
                                         t.ws[1][:, k * P:(k + 1) * P],
                                         xs[:, MT + k, c],
                                         start=(k == 0), stop=False)
                        nc.tensor.matmul(t.B[:, c],
                                         t.ws[2][:, k * P:(k + 1) * P],
                                         xs[:, k, c],
                                         start=(k == 0), stop=False)
                    f16_chain(t, t.K1, 0, xsum, 0, True)
                else:
                    f16_chain(t, t.A, 1, xs, MT, True)
                    f16_chain(t, t.B, 2, xs, 0, True)
                    f16_chain(t, t.K1, 0, xsum, 0, True)

            def rec_and_combine(t):
                c, m = t.cols, t.m
                if t.kind == "r":
                    dr_chain(t, t.A, 4, h8, MT, False)
                    dr_chain(t, t.K1, 3, hsum8, 0, False)
                else:
                    mv, sm = (rh, rhsum) if t.kind == "h" else (hs, hsum)
                    f16_chain(t, t.A, 4, mv, MT, False)
                    f16_chain(t, t.K1, 3, sm, 0, False)
                # ALU ops may read only one PSUM operand: stage K1 in SBUF
                # (also releases its bank early)
                k1sb = act.tile([P, N], F32, tag="k1")
                nc.scalar.copy(k1sb[:, c], t.K1[:, c])
                pre_r = act.tile([P, N], F16, tag="pre")
                nc.vector.scalar_tensor_tensor(
                    pre_r[:, c], k1sb[:, c], 1.0, t.A[:, c],
                    op0=OP.mult, op1=OP.add)
                if t.kind == "r":
                    dr_chain(t, t.B, 5, h8, 0, False)
                else:
                    f16_chain(t, t.B, 5, mv, 0, False)
                pre_i = act.tile([P, N], F16, tag="pre")
                for ch in ((slice(0, N // 2), slice(N // 2, N))
                           if t.tail else (c,)):
                    nc.vector.scalar_tensor_tensor(
                        pre_i[:, ch], k1sb[:, ch], 1.0, t.B[:, ch],
                        op0=OP.mult, op1=OP.add)

                if t.kind == "z":
                    for col, pre in ((m, pre_r), (MT + m, pre_i)):
                        nc.scalar.activation(zs[:, col, c], pre[:, c], AF.Relu,
                                             bias=bz_sb[:, col:col + 1],
                                             scale=0.2)
                elif t.kind == "r":
                    for col, pre in ((m, pre_r), (MT + m, pre_i)):
                        rr = act.tile([P, N], F16, tag="rr")
                        nc.scalar.activation(rr[:, c], pre[:, c], AF.Relu,
                                             bias=br_sb[:, col:col + 1],
                                             scale=0.2 / S8)
                        nc.vector.scalar_tensor_tensor(
                            rh[:, col, c], rr[:, c], 1.0, hs[:, col, c],
                            op0=OP.min, op1=OP.mult)
                    nc.vector.tensor_tensor(rhsum[:, m, c], rh[:, m, c],
                                            rh[:, MT + m, c], OP.add)
                else:
                    for col, pre in ((m, pre_r), (MT + m, pre_i)):
                        # chunk the very last combine so its serial chain
                        # pipelines across vector/gpsimd/scalar
                        if t.tail and col >= MT:
                            chunks = (slice(0, N // 2), slice(N // 2, N))
                        else:
                            chunks = (c,)
                        for ci, ch in enumerate(chunks):
                            eng = nc.vector
                            t_ = act.tile([P, N], F16, tag="t")
                            nc.scalar.activation(t_[:, ch], pre[:, ch],
                                                 AF.Tanh,
                                                 bias=bh_sb[:, col:col + 1])
                            d = act.tile([P, N], F16, tag="d")
                            eng.scalar_tensor_tensor(
                                d[:, ch], t_[:, ch], -1.0, hs[:, col, ch],
                                op0=OP.mult, op1=OP.add)
                            e = act.tile([P, N], F16, tag="e")
                            eng.scalar_tensor_tensor(
                                e[:, ch], zs[:, col, ch], 1.0, d[:, ch],
                                op0=OP.min, op1=OP.mult)
                            o = act.tile([P, N], F16, tag="o")
                            eng.tensor_tensor(o[:, ch], e[:, ch],
                                              t_[:, ch], OP.add)
                            nc.sync.dma_start(oT[col * P:(col + 1) * P, ch],
                                              o[:, ch])

            full = slice(0, N)
            plan = ([("z", m) for m in range(4)]
                    + [("r", m) for m in range(MT)]
                    + [("z", m) for m in range(4, MT)]
                    + [("h", m) for m in range(MT)])

            prev = None
            for kind, m in plan:
                if kind == "r" and m == 0:
                    emit_fp8_casts()
                pre_ws = ws0 if (kind, m) == ("z", 0) else (
                    ws1 if (kind, m) == ("z", 1) else None)
                t = T(kind, m, full, ws=pre_ws,
                      tail=(kind == "h" and m == MT - 1))
                in_half(t, kmajor=(prev is None))
                if prev is not None:
                    rec_and_combine(prev)
                prev = t
            rec_and_combine(prev)

    nc.compile()
    return nc


def _tiles(mat):
    # (1024, 1024) -> [m, k, 128, 128] tile array
    return mat.reshape(KT, P, MT, P).transpose(2, 0, 1, 3)


def _gate_mats(real_kernel, imaginary_kernel, real_recurrent_kernel,
               imaginary_recurrent_kernel, g):
    def gate(Wmat):
        return np.asarray(Wmat[:, g * U:(g + 1) * U], dtype=np.float32)

    R, I = gate(real_kernel), gate(imaginary_kernel)
    RR, IR = gate(real_recurrent_kernel), gate(imaginary_recurrent_kernel)
    return [R, I - R, -(I + R), RR, IR - RR, -(IR + RR)]


def _blob16(mats):
    arr = np.stack([_tiles(m) for m in mats])   # [6, m, k, p, c]
    arr = arr.transpose(1, 0, 3, 2, 4)          # [m, mat, p, k, c]
    return arr.reshape(MT, 6, P, MCOLS).astype(np.float16)


def _blob8(mats):
    arr = np.stack([_tiles(m) for m in mats])
    arr = arr.transpose(1, 0, 3, 2, 4) * S8     # [m, mat, p, k, c]
    return np.ascontiguousarray(arr).astype(ml_dtypes.float8_e4m3)


def prepare_in_maps(inputs, h_tm1, real_kernel, imaginary_kernel,
                    real_recurrent_kernel, imaginary_recurrent_kernel,
                    real_bias, imaginary_bias):
    inputs = np.asarray(inputs, dtype=np.float32)
    h_tm1 = np.asarray(h_tm1, dtype=np.float32)

    def mats(g):
        return _gate_mats(real_kernel, imaginary_kernel,
                          real_recurrent_kernel, imaginary_recurrent_kernel, g)

    w16_np = np.ascontiguousarray(np.stack([_blob16(mats(0)),
                                            _blob16(mats(2))]))
    w8_np = _blob8(mats(1))

    def cat_bias(g):
        return np.concatenate([
            np.asarray(real_bias[g * U:(g + 1) * U], dtype=np.float32),
            np.asarray(imaginary_bias[g * U:(g + 1) * U], dtype=np.float32),
        ])

    def pcols(v):  # [2U] -> [P, 2MT]
        return np.ascontiguousarray(v.reshape(2 * MT, P).T)

    bz_np = pcols(0.2 * cat_bias(0) + 0.5)
    br_np = pcols(0.2 * cat_bias(1) + 0.5)
    bh_np = pcols(cat_bias(2))

    in_maps = []
    for c in range(NCORES):
        sl = slice(c * N, (c + 1) * N)
        in_maps.append({
            "xT": inputs[sl].T.astype(np.float16),
            "hT": h_tm1[sl].T.astype(np.float16),
            "w16": w16_np, "w8": w8_np,
            "bz": bz_np, "br": br_np, "bh": bh_np,
        })
    return in_maps


def get_nc():
    if "nc" not in _CACHE:
        _CACHE["nc"] = _build()
    return _CACHE["nc"]


def gather(results):
    out = np.empty((B, F), dtype=np.float32)
    for c in range(NCORES):
        out[c * N:(c + 1) * N] = results[c]["oT"].T.astype(np.float32)
    return out


def kernel(**inputs):
    nc = get_nc()
    in_maps = prepare_in_maps(**inputs)
    res = run_bass_kernel_spmd(nc, in_maps, list(range(NCORES)))
    return gather(res.results)
